# revision 28
# baseline (speedup 1.0000x reference)
"""Trainium2 Bass kernel for the MoE routing module (nn_MoE_53042846105633).

Strategy: top-2 sparse dispatch, expert-parallel across 8 NeuronCores.
The router (0.8% of reference FLOPs) runs on host in fp64 during input
sharding -- the sharding_hint's "dispatch of tokens by top-k expert id"
-- which cuts device matmul work 4x vs dense: only the 2048 routed
(token, expert) pairs are computed instead of 8 experts x 1024 tokens.

Token counts per expert are skewed (e.g. [199,263,548,380,142,99,271,146]
for the seed-0 data), so each expert's 16 hidden tiles are split into two
8-tile halves and the 16 halves are packed onto cores as (big, small)
slot pairs: slot0 capacity C0 = largest expert count, slot1 capacity C1 =
5th-largest. All 8 cores run the identical instruction stream (SPMD);
per-core data (which expert half + which token columns) differs. With 16
half-slots and >=2 slots per expert this (C0 + C1) packing is optimal.

Per core: mm1 = 8 hid-tiles x 24 K-chunks over C0 cols + 8 x 24 over C1
(~60us PE at bf16), relu -> eh bf16, mm2 contracts eh against eW2 per
128-token stationary slice, bias + per-token routing weight (0 on padding)
-> weighted partial rows DMA'd out; host scatter-adds them into [B, 10].
Weights stream from HBM (12.6MB/core) well under the mm1 time. No
collectives (a NEFF with collectives drops the PE to 2.0 GHz). Expert
math is plain bf16 (fp32 PSUM accumulate), ~3e-3 relative output error;
the router/top-2/combine weights are exact (host fp64).

Schedule (measured ~83us exec): NEFF preamble + DMA-engine init means
first data lands ~11us in; HAM-warm matmuls hold the PE clock at 2.4GHz
until then. The first 4 slot0 tiles run jointly k-outer, their matmuls
emitted in predicted-data-arrival order (ramp-tile weights stream as
halves with subtile deps so work starts ~2us sooner); the DMA queue is
FIFO so emission order is tuned to keep the PE dense from ~13us on. All
16 ew1 tiles stay SBUF-resident (a tile-reuse WAR dep would stall the
whole FIFO queue). Slot1's mm2 pre-accumulates half its contraction
before the last mm1 tile and leaves via one merged out-DMA, shortening
the tail to ~2us after the last matmul.
"""

import sys

sys.path.insert(0, "/opt/trn_rl_repo")

import numpy as np
import ml_dtypes

BF16 = ml_dtypes.bfloat16

# Model dims (fixed for this problem)
B = 1024          # tokens
DIN = 3072        # input features
RHID = 128        # router hidden
E = 8             # experts = cores
EHID = 2048       # expert hidden
NCLS = 10         # classes
KC1 = DIN // 128  # 24 K-chunks for DIN contraction
KC2 = EHID // 128 # 16 hid tiles per expert
HK = KC2 // 2     # 8 hid tiles per half-slot
TOPK = 2

_PROGRAMS = {}
LAST_RESULTS = None


def _ensure_axon_profile_hook():
    """bass_utils' trace=True path imports antenv.axon_hooks, which this
    image lacks. Provide it (backed by libaxon_pjrt.so's NRT profile C API)
    so NTFF profiling works; degrade silently if unavailable."""
    import contextlib
    import ctypes
    import os
    import types

    try:
        from antenv.axon_hooks import get_axon_ntff_profile_hook  # noqa: F401
        return
    except ImportError:
        pass
    try:
        import antenv
    except ImportError:
        return

    state = {"hook": None}
    mod = types.ModuleType("antenv.axon_hooks")
    mod.set_axon_ntff_profile_hook = lambda h: state.__setitem__("hook", h)
    mod.get_axon_ntff_profile_hook = lambda: state["hook"]
    sys.modules["antenv.axon_hooks"] = mod
    antenv.axon_hooks = mod

    so_path = "/opt/axon/libaxon_pjrt.so"
    if not os.path.exists(so_path):
        return
    try:
        lib = ctypes.CDLL(so_path)
    except OSError:
        return
    if not hasattr(lib, "axon_start_nrt_profile"):
        return
    lib.axon_start_nrt_profile.argtypes = [
        ctypes.POINTER(ctypes.c_int64), ctypes.c_size_t]
    lib.axon_start_nrt_profile.restype = ctypes.c_int64
    lib.axon_stop_nrt_profile.argtypes = [ctypes.c_char_p]
    lib.axon_stop_nrt_profile.restype = ctypes.c_int64

    @contextlib.contextmanager
    def _hook(output_dir, device_ids):
        import jax

        jax.devices()
        if device_ids:
            ids = (ctypes.c_int64 * len(device_ids))(*device_ids)
            rc = lib.axon_start_nrt_profile(ids, len(device_ids))
        else:
            rc = lib.axon_start_nrt_profile(None, 0)
        if rc != 0:
            raise RuntimeError(f"axon_start_nrt_profile rc={rc}")
        try:
            yield
        finally:
            n = lib.axon_stop_nrt_profile(str(output_dir).encode())
            print(f"profile: {n} ntff file(s) -> {output_dir}",
                  file=sys.stderr)

    state["hook"] = _hook


def _chunks(c):
    """Split c columns into <=512 PSUM-bank-sized pieces (each >=128 wide
    when possible so matmuls stay moving-bound)."""
    n = -(-c // 512)
    base = c // n
    out = []
    off = 0
    for i in range(n):
        w = base + (1 if i < c % n else 0)
        out.append((off, off + w))
        off += w
    return out


def _build_program(C0, C1):
    import concourse.tile as tile
    from concourse import bacc, mybir

    f32 = mybir.dt.float32
    bf = mybir.dt.bfloat16
    AF = mybir.ActivationFunctionType
    ALU = mybir.AluOpType

    caps = [C0] * HK + [C1] * HK      # columns per hid tile
    pads = [-(-C0 // 128) * 128, -(-C1 // 128) * 128]
    ntt = pads[0] // 128 + pads[1] // 128   # total 128-token out tiles
    RAMP = 4                          # slot0 tiles run k-outer in the ramp

    nc = bacc.Bacc("TRN2", debug=False, num_devices=E)

    # ---- DRAM I/O (all per-core data) ---------------------------------
    # x: [i, k, n]; slot0 tokens streamed per-k (paces the ramp k-outer),
    # slot1 tokens as one late transfer (first needed ~halfway through)
    d_x0 = nc.dram_tensor("x0", [128, KC1, C0], bf, kind="ExternalInput")
    d_x1 = nc.dram_tensor("x1", [128, KC1, C1], bf, kind="ExternalInput")
    # stationary tiles: t<8 slot0 expert half, t>=8 slot1 expert half
    # layout [t, i, (k j)]: element = eW1[e][128k + i, 128*m(t) + j]
    d_ew1 = nc.dram_tensor("ew1", [KC2, 128, DIN], bf, kind="ExternalInput")
    d_eb1 = nc.dram_tensor("eb1", [128, KC2], f32, kind="ExternalInput")
    # ew2[:, t, :] = eW2[e][128*m(t):128*(m(t)+1), :]
    d_ew2 = nc.dram_tensor("ew2", [128, KC2, NCLS], bf, kind="ExternalInput")
    # eb2 tiled to 128 partitions per slot
    d_eb2 = nc.dram_tensor("eb2", [128, 2, NCLS], f32, kind="ExternalInput")
    # per-token normalized routing weight, [token-in-tile, out-tile]
    d_w = nc.dram_tensor("w", [128, ntt], f32, kind="ExternalInput")
    # weighted partial rows; host scatter-adds real rows into [B, NCLS]
    d_out = nc.dram_tensor("out", [ntt * 128, NCLS], f32,
                           kind="ExternalOutput")

    with tile.TileContext(nc) as tc:
        with (
            tc.tile_pool(name="const", bufs=1) as cp,
            tc.tile_pool(name="wstream", bufs=16) as wp,
            tc.tile_pool(name="psum", bufs=1, space="PSUM") as pp,
            tc.tile_pool(name="outp", bufs=1) as op,
        ):
            # ---- HAM pre-warm: full-array K=128 dummies flip the PE
            # clock gate to 2.4 GHz during the DMA ramp (small-K matmuls
            # don't count as PE-busy).
            warmt = cp.tile([128, 128], bf, tag="warmt", name="warmt")
            nc.vector.memset(warmt[:], 1.0)
            warm = pp.tile([128, 128], f32, tag="mm1", bufs=8, name="warm")
            for _i in range(48):
                nc.tensor.matmul(warm[:], warmt[:], warmt[:],
                                 start=True, stop=True)

            # eh tiles, padded to 128-multiples; zero the pad columns so
            # mm2 stationary slices never read uninitialized SBUF.
            ehs = []
            for t in range(KC2):
                g = t // HK
                eh = cp.tile([128, pads[g]], bf, tag=f"eh{t}", name=f"eh{t}")
                ehs.append(eh)
                cap = caps[t]
                if cap < pads[g]:
                    nc.vector.memset(eh[:, cap:], 0.0)

            # ---- input DMA (emission order = FIFO queue order) --------
            # All 16 ew1 tiles are SBUF-resident (no WAR queue stalls);
            # slot0 x chunks pace the ramp k-outer; everything slot1
            # needs comes after the slot0-critical stream.
            wts = {}

            def load_ew1(t):
                wt = wp.tile([128, DIN], bf, tag="ew1", name=f"ew1t{t}")
                nc.sync.dma_start(wt[:, :DIN // 2], d_ew1[t][:, :DIN // 2])
                nc.sync.dma_start(wt[:, DIN // 2:], d_ew1[t][:, DIN // 2:])
                wts[t] = wt

            # Arrival model (300 GB/s early-sustained, first packet
            # ~9.3us) used to order the ramp's matmul emission by data
            # readiness; calibrated against measured traces.
            arr = {}
            cum = [9300.0]

            def track(key, nbytes):
                cum[0] += nbytes / 0.30e3
                arr[key] = cum[0]

            eb1t = cp.tile([128, KC2], f32, tag="eb1", name="eb1t")
            nc.sync.dma_start(eb1t[:], d_eb1[:])
            track("eb1", KC2 * 512)
            x0k = []
            for k in range(KC1):
                t = cp.tile([128, C0], bf, tag=f"x0k{k}", name=f"x0k{k}")
                x0k.append(t)

            def load_x0(k):
                nc.sync.dma_start(x0k[k][:], d_x0[:, k, :])
                track(("x0", k), 128 * C0 * 2)

            # ramp tiles stream as halves with subtile deps: k<12 matmuls
            # only wait on the first half, so real work starts ~4us sooner
            def load_ew1_half(t, h):
                if t not in wts:
                    wts[t] = wp.tile([128, DIN], bf, tag="ew1",
                                     name=f"ew1t{t}")
                lo, hi = (0, DIN // 2) if h == 0 else (DIN // 2, DIN)
                nc.sync.dma_start(wts[t][:, lo:hi], d_ew1[t][:, lo:hi])
                track(("ew1", t, h), 128 * (DIN // 2) * 2)

            load_ew1_half(0, 0)
            load_x0(0)
            load_ew1_half(1, 0)
            load_x0(1)
            load_x0(2)
            load_ew1_half(2, 0)
            load_x0(3)
            load_x0(4)
            load_ew1_half(3, 0)
            for k in range(5, 13):
                load_x0(k)
            load_ew1_half(0, 1)
            load_x0(13)
            load_x0(14)
            load_ew1_half(1, 1)
            for k in range(15, 18):
                load_x0(k)
            load_ew1_half(2, 1)
            for k in range(18, 21):
                load_x0(k)
            load_ew1_half(3, 1)
            for k in range(21, KC1):
                load_x0(k)
            for t in range(4, HK):
                load_ew1(t)
            ew2t = cp.tile([128, KC2, NCLS], bf, tag="ew2", name="ew2t")
            nc.sync.dma_start(ew2t[:], d_ew2[:])
            eb2t = cp.tile([128, 2, NCLS], f32, tag="eb2", name="eb2t")
            nc.sync.dma_start(eb2t[:], d_eb2[:])
            wtok = cp.tile([128, ntt], f32, tag="wtok", name="wtok")
            nc.sync.dma_start(wtok[:], d_w[:])
            load_ew1(8)
            x1t = cp.tile([128, KC1, C1], bf, tag="x1t", name="x1t")
            nc.sync.dma_start(x1t[:], d_x1[:])
            for t in range(9, KC2):
                load_ew1(t)

            pos_g1 = {}

            def emit_mm2_g0():
                # out[tok, cls]: psum rows = tokens, stationary = 128-token
                # slices of this slot's eh tiles.
                for tt in range(pads[0] // 128):
                    po = pp.tile([128, NCLS], f32, tag="mm1", bufs=8,
                                 name=f"po0_{tt}")
                    for j in range(HK):
                        nc.tensor.matmul(
                            po[:],
                            ehs[j][:, tt * 128:(tt + 1) * 128],
                            ew2t[:, j, :],
                            start=(j == 0),
                            stop=(j == HK - 1),
                        )
                    osb = op.tile([128, NCLS], f32, tag="osb", bufs=4,
                                  name=f"osb0_{tt}")
                    nc.vector.tensor_add(osb[:], po[:], eb2t[:, 0, :])
                    nc.vector.tensor_scalar(
                        osb[:], osb[:], wtok[:, tt:tt + 1], None, ALU.mult)
                    nc.sync.dma_start(
                        d_out[tt * 128:(tt + 1) * 128, :], osb[:])

            def emit_mm2_g1(js, finish):
                # slot1 mm2, split so only `js` of the contraction runs
                # after the last mm1 tile; one merged out DMA at the end
                # (rows interleaved as base + ntt1*token + tt; the host
                # scatter indexes accordingly).
                base = pads[0] // 128
                ntt1 = pads[1] // 128
                for tt in range(ntt1):
                    po = pos_g1.get(tt)
                    if po is None:
                        po = pp.tile([128, NCLS], f32, tag="mm1", bufs=8,
                                     name=f"po1_{tt}")
                        pos_g1[tt] = po
                    for j in js:
                        nc.tensor.matmul(
                            po[:],
                            ehs[HK + j][:, tt * 128:(tt + 1) * 128],
                            ew2t[:, HK + j, :],
                            start=(j == 0),
                            stop=(finish and j == js[-1]),
                        )
                if not finish:
                    return
                osb1 = op.tile([128, ntt1, NCLS], f32, tag="osb1",
                               name="osb1")
                for tt in range(ntt1):
                    nc.vector.tensor_add(osb1[:, tt, :], pos_g1[tt][:],
                                         eb2t[:, 1, :])
                    nc.vector.tensor_scalar(
                        osb1[:, tt, :], osb1[:, tt, :],
                        wtok[:, base + tt:base + tt + 1], None, ALU.mult)
                nc.sync.dma_start(d_out[base * 128:, :], osb1[:])

            # ---- mm1: eh[t] = relu(W1_tile[t].T @ x_cols + b) ----------
            def xsrc(t, k, a, b):
                return (x0k[k][:, a:b] if t < HK
                        else x1t[:, k, a:b])

            ch0 = _chunks(C0)
            # ramp: first RAMP slot0 tiles jointly k-outer; cells (t, k)
            # emitted in predicted-data-arrival order so the PE never
            # idles (idle gaps also drop the HAM clock below 2.4 GHz)
            ramp_pss = {t: [pp.tile([128, b - a], f32, tag="mm1", bufs=8,
                                    name=f"ps1_{t}_{i}")
                            for i, (a, b) in enumerate(ch0)]
                        for t in range(RAMP)}
            cells = sorted(
                ((max(arr[("ew1", t, 0 if k < KC1 // 2 else 1)],
                      arr[("x0", k)]), k, t)
                 for t in range(RAMP) for k in range(KC1)),
                key=lambda c: (c[0], c[1], c[2]))
            seen = {t: 0 for t in range(RAMP)}
            for _, k, t in cells:
                seen[t] += 1
                for i, (a, b) in enumerate(ch0):
                    nc.tensor.matmul(
                        ramp_pss[t][i][:],
                        wts[t][:, k * 128:(k + 1) * 128],
                        x0k[k][:, a:b],
                        start=(seen[t] == 1),
                        stop=(seen[t] == KC1),
                    )
            for t in range(RAMP):
                for i, (a, b) in enumerate(ch0):
                    nc.scalar.activation(
                        ehs[t][:, a:b], ramp_pss[t][i][:],
                        AF.Relu, bias=eb1t[:, t:t + 1],
                    )

            for t in range(RAMP, KC2):
                wt = wts[t]
                cap = caps[t]
                ch = _chunks(cap)
                pss = [pp.tile([128, b - a], f32, tag="mm1", bufs=8,
                               name=f"ps1_{t}_{i}") for i, (a, b) in
                       enumerate(ch)]
                for i, (a, b) in enumerate(ch):
                    for k in range(KC1):
                        nc.tensor.matmul(
                            pss[i][:],
                            wt[:, k * 128:(k + 1) * 128],
                            xsrc(t, k, a, b),
                            start=(k == 0),
                            stop=(k == KC1 - 1),
                        )
                for i, (a, b) in enumerate(ch):
                    nc.scalar.activation(
                        ehs[t][:, a:b], pss[i][:],
                        AF.Relu, bias=eb1t[:, t:t + 1],
                    )
                if t == HK + 1:
                    # slot0's eh tiles are complete; its mm2 hides under
                    # the remaining slot1 mm1 work
                    emit_mm2_g0()
                if t == KC2 - 5:
                    # pre-accumulate the first half of slot1's mm2 so
                    # only 4 matmuls per out-tile trail the last mm1 tile
                    emit_mm2_g1(list(range(4)), finish=False)
            emit_mm2_g1(list(range(4, HK)), finish=True)

    return nc


def _route_host(x, rW1, rb1, rW2, rb2):
    """Exact router on host: top-2 expert ids + normalized weights."""
    xf = np.asarray(x, np.float64).reshape(B, DIN)
    rh = np.maximum(xf @ np.asarray(rW1, np.float64)
                    + np.asarray(rb1, np.float64), 0.0)
    logits = rh @ np.asarray(rW2, np.float64) + np.asarray(rb2, np.float64)
    m = logits.max(-1, keepdims=True)
    p = np.exp(logits - m)
    p /= p.sum(-1, keepdims=True)
    idx = np.argsort(-p, axis=-1, kind="stable")[:, :TOPK]
    w = np.take_along_axis(p, idx, axis=-1)
    w /= w.sum(-1, keepdims=True)
    return idx, w


def _plan(counts):
    """Pack the 16 expert-halves into 8 (slot0, slot1) core pairs.

    Returns (C0, C1, plan) with plan[c] = [expert_slot0, expert_slot1];
    core c takes half c%2 of each. Slot0 = the 4 largest experts, slot1 =
    the 4 smallest: provably minimal C0 + C1 for this slot structure.
    """
    order = np.argsort(-counts, kind="stable")
    C0 = int(counts[order[0]])
    C1 = max(int(counts[order[4]]), 1)
    plan = []
    for c in range(E):
        plan.append((int(order[c // 2]), int(order[4 + c // 2])))
    return C0, C1, plan


def _prep_inputs(x, rW1, rb1, rW2, rb2, eW1, eb1, eW2, eb2):
    """Host routing + shard/layout prep. Returns (in_maps, scatter, ntt)."""
    idx, w = _route_host(x, rW1, rb1, rW2, rb2)
    tok_of = [np.where((idx == e).any(-1))[0] for e in range(E)]
    w_of = [w[(t := tok_of[e]), (idx[t] == e).argmax(-1)]
            for e in range(E)]
    counts = np.array([len(t) for t in tok_of])
    C0, C1, plan = _plan(counts)

    xf = np.ascontiguousarray(np.asarray(x, np.float32).reshape(B, DIN))
    # [i, k, n] layout: xt[:, k, t] = xf[t, 128k + i]
    xt = xf.reshape(B, KC1, 128).transpose(2, 1, 0).astype(BF16)

    ew1_full = [np.asarray(eW1[e], np.float32)
                .reshape(KC1, 128, KC2, 128)
                .transpose(2, 1, 0, 3)
                .reshape(KC2, 128, DIN)
                .astype(BF16) for e in range(E)]
    eb1_full = [np.asarray(eb1[e], np.float32).reshape(KC2, 128).T
                for e in range(E)]
    ew2_full = [np.asarray(eW2[e], np.float32)
                .reshape(KC2, 128, NCLS)
                .transpose(1, 0, 2)
                .astype(BF16) for e in range(E)]

    pads = [-(-C0 // 128) * 128, -(-C1 // 128) * 128]
    ntt = (pads[0] + pads[1]) // 128
    caps = [C0, C1]

    in_maps = []
    scatter = []   # per core: (row_offset, token_ids) per slot
    for c in range(E):
        xblob = [np.zeros((128, KC1, C0), BF16),
                 np.zeros((128, KC1, C1), BF16)]
        ew1b = np.empty((KC2, 128, DIN), BF16)
        eb1b = np.empty((128, KC2), np.float32)
        ew2b = np.empty((128, KC2, NCLS), BF16)
        eb2b = np.empty((128, 2, NCLS), np.float32)
        wb = np.zeros((128, ntt), np.float32)
        half = c % 2
        sc = []
        for g in range(2):
            e = plan[c][g]
            toks = tok_of[e]
            n = len(toks)
            xblob[g][:, :, :n] = xt[:, :, toks]
            mlo = half * HK
            ew1b[g * HK:(g + 1) * HK] = ew1_full[e][mlo:mlo + HK]
            eb1b[:, g * HK:(g + 1) * HK] = eb1_full[e][:, mlo:mlo + HK]
            ew2b[:, g * HK:(g + 1) * HK] = ew2_full[e][:, mlo:mlo + HK]
            eb2b[:, g, :] = np.asarray(eb2[e], np.float32)[None, :]
            base_tt = 0 if g == 0 else pads[0] // 128
            wcol = np.zeros(pads[g], np.float32)
            wcol[:n] = w_of[e]
            wb[:, base_tt:base_tt + pads[g] // 128] = (
                wcol.reshape(-1, 128).T)
            j = np.arange(n)
            if g == 0:
                rows = base_tt * 128 + j
            else:
                # slot1 partials leave the core via one merged DMA with
                # rows interleaved as ntt1*token + tile
                ntt1 = pads[1] // 128
                rows = base_tt * 128 + (j % 128) * ntt1 + j // 128
            sc.append((rows, toks))
            assert n <= caps[g]
        in_maps.append({
            "x0": np.ascontiguousarray(xblob[0]),
            "x1": np.ascontiguousarray(xblob[1]),
            "ew1": np.ascontiguousarray(ew1b),
            "eb1": eb1b, "ew2": np.ascontiguousarray(ew2b),
            "eb2": eb2b, "w": wb,
        })
        scatter.append(sc)
    return in_maps, scatter, C0, C1


def kernel(x, rW1, rb1, rW2, rb2, eW1, eb1, eW2, eb2):
    global LAST_RESULTS
    _ensure_axon_profile_hook()
    from concourse.bass_utils import run_bass_kernel_spmd

    in_maps, scatter, C0, C1 = _prep_inputs(
        x, rW1, rb1, rW2, rb2, eW1, eb1, eW2, eb2)
    key = (C0, C1)
    nc = _PROGRAMS.get(key)
    if nc is None:
        nc = _build_program(C0, C1)
        nc.finalize()
        _PROGRAMS[key] = nc
    res = run_bass_kernel_spmd(nc, in_maps, core_ids=list(range(E)))
    LAST_RESULTS = res
    out = np.zeros((B, NCLS), np.float32)
    for c, r in enumerate(res.results):
        part = np.asarray(r["out"], np.float32)
        for (rows, toks) in scatter[c]:
            np.add.at(out, toks, part[rows])
    return out


# revision 29
# speedup vs baseline: 1.0215x; 1.0215x over previous
"""Trainium2 Bass kernel for the MoE routing module (nn_MoE_53042846105633).

Strategy: top-2 sparse dispatch, expert-parallel across 8 NeuronCores.
The router (0.8% of reference FLOPs) runs on host in fp64 during input
sharding -- the sharding_hint's "dispatch of tokens by top-k expert id"
-- which cuts device matmul work 4x vs dense: only the 2048 routed
(token, expert) pairs are computed instead of 8 experts x 1024 tokens.

Token counts per expert are skewed (e.g. [199,263,548,380,142,99,271,146]
for the seed-0 data), so each expert's 16 hidden tiles are split into two
8-tile halves and the 16 halves are packed onto cores as (big, small)
slot pairs: slot0 capacity C0 = largest expert count, slot1 capacity C1 =
5th-largest. All 8 cores run the identical instruction stream (SPMD);
per-core data (which expert half + which token columns) differs. With 16
half-slots and >=2 slots per expert this (C0 + C1) packing is optimal.

Per core: mm1 = 8 hid-tiles x 24 K-chunks over C0 cols + 8 x 24 over C1
(~60us PE at bf16), relu -> eh bf16, mm2 contracts eh against eW2 per
128-token stationary slice, bias + per-token routing weight (0 on padding)
-> weighted partial rows DMA'd out; host scatter-adds them into [B, 10].
Weights stream from HBM (12.6MB/core) well under the mm1 time. No
collectives (a NEFF with collectives drops the PE to 2.0 GHz). Expert
math is plain bf16 (fp32 PSUM accumulate), ~3e-3 relative output error;
the router/top-2/combine weights are exact (host fp64).

Schedule (measured ~83us exec): NEFF preamble + DMA-engine init means
first data lands ~11us in; HAM-warm matmuls hold the PE clock at 2.4GHz
until then. The first 4 slot0 tiles run jointly k-outer, their matmuls
emitted in predicted-data-arrival order (ramp-tile weights stream as
halves with subtile deps so work starts ~2us sooner); the DMA queue is
FIFO so emission order is tuned to keep the PE dense from ~13us on. All
16 ew1 tiles stay SBUF-resident (a tile-reuse WAR dep would stall the
whole FIFO queue). Slot1's mm2 pre-accumulates half its contraction
before the last mm1 tile and leaves via one merged out-DMA, shortening
the tail to ~2us after the last matmul.
"""

import sys

sys.path.insert(0, "/opt/trn_rl_repo")

import numpy as np
import ml_dtypes

BF16 = ml_dtypes.bfloat16

# Model dims (fixed for this problem)
B = 1024          # tokens
DIN = 3072        # input features
RHID = 128        # router hidden
E = 8             # experts = cores
EHID = 2048       # expert hidden
NCLS = 10         # classes
KC1 = DIN // 128  # 24 K-chunks for DIN contraction
KC2 = EHID // 128 # 16 hid tiles per expert
HK = KC2 // 2     # 8 hid tiles per half-slot
TOPK = 2

_PROGRAMS = {}
LAST_RESULTS = None


def _ensure_axon_profile_hook():
    """bass_utils' trace=True path imports antenv.axon_hooks, which this
    image lacks. Provide it (backed by libaxon_pjrt.so's NRT profile C API)
    so NTFF profiling works; degrade silently if unavailable."""
    import contextlib
    import ctypes
    import os
    import types

    try:
        from antenv.axon_hooks import get_axon_ntff_profile_hook  # noqa: F401
        return
    except ImportError:
        pass
    try:
        import antenv
    except ImportError:
        return

    state = {"hook": None}
    mod = types.ModuleType("antenv.axon_hooks")
    mod.set_axon_ntff_profile_hook = lambda h: state.__setitem__("hook", h)
    mod.get_axon_ntff_profile_hook = lambda: state["hook"]
    sys.modules["antenv.axon_hooks"] = mod
    antenv.axon_hooks = mod

    so_path = "/opt/axon/libaxon_pjrt.so"
    if not os.path.exists(so_path):
        return
    try:
        lib = ctypes.CDLL(so_path)
    except OSError:
        return
    if not hasattr(lib, "axon_start_nrt_profile"):
        return
    lib.axon_start_nrt_profile.argtypes = [
        ctypes.POINTER(ctypes.c_int64), ctypes.c_size_t]
    lib.axon_start_nrt_profile.restype = ctypes.c_int64
    lib.axon_stop_nrt_profile.argtypes = [ctypes.c_char_p]
    lib.axon_stop_nrt_profile.restype = ctypes.c_int64

    @contextlib.contextmanager
    def _hook(output_dir, device_ids):
        import jax

        jax.devices()
        if device_ids:
            ids = (ctypes.c_int64 * len(device_ids))(*device_ids)
            rc = lib.axon_start_nrt_profile(ids, len(device_ids))
        else:
            rc = lib.axon_start_nrt_profile(None, 0)
        if rc != 0:
            raise RuntimeError(f"axon_start_nrt_profile rc={rc}")
        try:
            yield
        finally:
            n = lib.axon_stop_nrt_profile(str(output_dir).encode())
            print(f"profile: {n} ntff file(s) -> {output_dir}",
                  file=sys.stderr)

    state["hook"] = _hook


def _chunks(c):
    """Split c columns into <=512 PSUM-bank-sized pieces (each >=128 wide
    when possible so matmuls stay moving-bound)."""
    n = -(-c // 512)
    base = c // n
    out = []
    off = 0
    for i in range(n):
        w = base + (1 if i < c % n else 0)
        out.append((off, off + w))
        off += w
    return out


def _build_program(C0, C1):
    import concourse.tile as tile
    from concourse import bacc, mybir

    f32 = mybir.dt.float32
    bf = mybir.dt.bfloat16
    AF = mybir.ActivationFunctionType
    ALU = mybir.AluOpType

    caps = [C0] * HK + [C1] * HK      # columns per hid tile
    pads = [-(-C0 // 128) * 128, -(-C1 // 128) * 128]
    ntt = pads[0] // 128 + pads[1] // 128   # total 128-token out tiles
    RAMP = 4                          # slot0 tiles run k-outer in the ramp

    nc = bacc.Bacc("TRN2", debug=False, num_devices=E)

    # ---- DRAM I/O (all per-core data) ---------------------------------
    # x: [i, k, n]; slot0 tokens streamed per-k (paces the ramp k-outer),
    # slot1 tokens as one late transfer (first needed ~halfway through)
    d_x0 = nc.dram_tensor("x0", [128, KC1, C0], bf, kind="ExternalInput")
    d_x1 = nc.dram_tensor("x1", [128, KC1, C1], bf, kind="ExternalInput")
    # stationary tiles: t<8 slot0 expert half, t>=8 slot1 expert half
    # layout [t, i, (k j)]: element = eW1[e][128k + i, 128*m(t) + j]
    d_ew1 = nc.dram_tensor("ew1", [KC2, 128, DIN], bf, kind="ExternalInput")
    d_eb1 = nc.dram_tensor("eb1", [128, KC2], f32, kind="ExternalInput")
    # ew2[:, t, :] = eW2[e][128*m(t):128*(m(t)+1), :]
    d_ew2 = nc.dram_tensor("ew2", [128, KC2, NCLS], bf, kind="ExternalInput")
    # eb2 tiled to 128 partitions per slot
    d_eb2 = nc.dram_tensor("eb2", [128, 2, NCLS], f32, kind="ExternalInput")
    # per-token normalized routing weight, [token-in-tile, out-tile]
    d_w = nc.dram_tensor("w", [128, ntt], f32, kind="ExternalInput")
    # weighted partial rows; host scatter-adds real rows into [B, NCLS]
    d_out = nc.dram_tensor("out", [ntt * 128, NCLS], f32,
                           kind="ExternalOutput")

    with tile.TileContext(nc) as tc:
        with (
            tc.tile_pool(name="const", bufs=1) as cp,
            tc.tile_pool(name="wstream", bufs=16) as wp,
            tc.tile_pool(name="psum", bufs=1, space="PSUM") as pp,
            tc.tile_pool(name="outp", bufs=1) as op,
        ):
            # ---- HAM pre-warm: full-array K=128 dummies flip the PE
            # clock gate to 2.4 GHz during the DMA ramp (small-K matmuls
            # don't count as PE-busy).
            warmt = cp.tile([128, 128], bf, tag="warmt", name="warmt")
            nc.vector.memset(warmt[:], 1.0)
            warm = pp.tile([128, 128], f32, tag="mm1", bufs=8, name="warm")
            for _i in range(38):
                nc.tensor.matmul(warm[:], warmt[:], warmt[:],
                                 start=True, stop=True)

            # eh tiles, padded to 128-multiples; zero the pad columns so
            # mm2 stationary slices never read uninitialized SBUF.
            ehs = []
            for t in range(KC2):
                g = t // HK
                eh = cp.tile([128, pads[g]], bf, tag=f"eh{t}", name=f"eh{t}")
                ehs.append(eh)
                cap = caps[t]
                if cap < pads[g]:
                    nc.vector.memset(eh[:, cap:], 0.0)

            # ---- input DMA (emission order = FIFO queue order) --------
            # All 16 ew1 tiles are SBUF-resident (no WAR queue stalls);
            # slot0 x chunks pace the ramp k-outer; everything slot1
            # needs comes after the slot0-critical stream.
            wts = {}

            def load_ew1(t):
                wt = wp.tile([128, DIN], bf, tag="ew1", name=f"ew1t{t}")
                nc.sync.dma_start(wt[:, :DIN // 2], d_ew1[t][:, :DIN // 2])
                nc.sync.dma_start(wt[:, DIN // 2:], d_ew1[t][:, DIN // 2:])
                wts[t] = wt

            # Arrival model (300 GB/s early-sustained, first packet
            # ~9.3us) used to order the ramp's matmul emission by data
            # readiness; calibrated against measured traces.
            arr = {}
            cum = [9300.0]

            def track(key, nbytes):
                cum[0] += nbytes / 0.30e3
                arr[key] = cum[0]

            eb1t = cp.tile([128, KC2], f32, tag="eb1", name="eb1t")
            nc.sync.dma_start(eb1t[:], d_eb1[:])
            track("eb1", KC2 * 512)
            x0k = []
            for k in range(KC1):
                t = cp.tile([128, C0], bf, tag=f"x0k{k}", name=f"x0k{k}")
                x0k.append(t)

            def load_x0(k):
                nc.sync.dma_start(x0k[k][:], d_x0[:, k, :])
                track(("x0", k), 128 * C0 * 2)

            # ramp tiles stream as halves with subtile deps: k<12 matmuls
            # only wait on the first half, so real work starts ~4us sooner
            def load_ew1_half(t, h):
                if t not in wts:
                    wts[t] = wp.tile([128, DIN], bf, tag="ew1",
                                     name=f"ew1t{t}")
                lo, hi = (0, DIN // 2) if h == 0 else (DIN // 2, DIN)
                nc.sync.dma_start(wts[t][:, lo:hi], d_ew1[t][:, lo:hi])
                track(("ew1", t, h), 128 * (DIN // 2) * 2)

            load_ew1_half(0, 0)
            load_x0(0)
            load_ew1_half(1, 0)
            load_x0(1)
            load_x0(2)
            load_ew1_half(2, 0)
            load_x0(3)
            load_x0(4)
            load_ew1_half(3, 0)
            for k in range(5, 13):
                load_x0(k)
            load_ew1_half(0, 1)
            load_x0(13)
            load_x0(14)
            load_ew1_half(1, 1)
            for k in range(15, 18):
                load_x0(k)
            load_ew1_half(2, 1)
            for k in range(18, 21):
                load_x0(k)
            load_ew1_half(3, 1)
            for k in range(21, KC1):
                load_x0(k)
            for t in range(4, HK):
                load_ew1(t)
            ew2t = cp.tile([128, KC2, NCLS], bf, tag="ew2", name="ew2t")
            nc.sync.dma_start(ew2t[:], d_ew2[:])
            eb2t = cp.tile([128, 2, NCLS], f32, tag="eb2", name="eb2t")
            nc.sync.dma_start(eb2t[:], d_eb2[:])
            wtok = cp.tile([128, ntt], f32, tag="wtok", name="wtok")
            nc.sync.dma_start(wtok[:], d_w[:])
            load_ew1(8)
            x1t = cp.tile([128, KC1, C1], bf, tag="x1t", name="x1t")
            nc.sync.dma_start(x1t[:], d_x1[:])
            for t in range(9, KC2):
                load_ew1(t)

            pos_g1 = {}

            def emit_mm2_g0():
                # out[tok, cls]: psum rows = tokens, stationary = 128-token
                # slices of this slot's eh tiles.
                for tt in range(pads[0] // 128):
                    po = pp.tile([128, NCLS], f32, tag="mm1", bufs=8,
                                 name=f"po0_{tt}")
                    for j in range(HK):
                        nc.tensor.matmul(
                            po[:],
                            ehs[j][:, tt * 128:(tt + 1) * 128],
                            ew2t[:, j, :],
                            start=(j == 0),
                            stop=(j == HK - 1),
                        )
                    osb = op.tile([128, NCLS], f32, tag="osb", bufs=4,
                                  name=f"osb0_{tt}")
                    nc.vector.tensor_add(osb[:], po[:], eb2t[:, 0, :])
                    nc.vector.tensor_scalar(
                        osb[:], osb[:], wtok[:, tt:tt + 1], None, ALU.mult)
                    nc.sync.dma_start(
                        d_out[tt * 128:(tt + 1) * 128, :], osb[:])

            def emit_mm2_g1(js, finish):
                # slot1 mm2, split so only `js` of the contraction runs
                # after the last mm1 tile; one merged out DMA at the end
                # (rows interleaved as base + ntt1*token + tt; the host
                # scatter indexes accordingly).
                base = pads[0] // 128
                ntt1 = pads[1] // 128
                for tt in range(ntt1):
                    po = pos_g1.get(tt)
                    if po is None:
                        po = pp.tile([128, NCLS], f32, tag="mm1", bufs=8,
                                     name=f"po1_{tt}")
                        pos_g1[tt] = po
                    for j in js:
                        nc.tensor.matmul(
                            po[:],
                            ehs[HK + j][:, tt * 128:(tt + 1) * 128],
                            ew2t[:, HK + j, :],
                            start=(j == 0),
                            stop=(finish and j == js[-1]),
                        )
                if not finish:
                    return
                osb1 = op.tile([128, ntt1, NCLS], f32, tag="osb1",
                               name="osb1")
                for tt in range(ntt1):
                    nc.vector.tensor_add(osb1[:, tt, :], pos_g1[tt][:],
                                         eb2t[:, 1, :])
                    nc.vector.tensor_scalar(
                        osb1[:, tt, :], osb1[:, tt, :],
                        wtok[:, base + tt:base + tt + 1], None, ALU.mult)
                nc.sync.dma_start(d_out[base * 128:, :], osb1[:])

            # ---- mm1: eh[t] = relu(W1_tile[t].T @ x_cols + b) ----------
            def xsrc(t, k, a, b):
                return (x0k[k][:, a:b] if t < HK
                        else x1t[:, k, a:b])

            ch0 = _chunks(C0)
            # ramp: first RAMP slot0 tiles jointly k-outer; cells (t, k)
            # emitted in predicted-data-arrival order so the PE never
            # idles (idle gaps also drop the HAM clock below 2.4 GHz)
            ramp_pss = {t: [pp.tile([128, b - a], f32, tag="mm1", bufs=8,
                                    name=f"ps1_{t}_{i}")
                            for i, (a, b) in enumerate(ch0)]
                        for t in range(RAMP)}
            cells = sorted(
                ((max(arr[("ew1", t, 0 if k < KC1 // 2 else 1)],
                      arr[("x0", k)]), k, t)
                 for t in range(RAMP) for k in range(KC1)),
                key=lambda c: (c[0], c[1], c[2]))
            seen = {t: 0 for t in range(RAMP)}
            for _, k, t in cells:
                seen[t] += 1
                for i, (a, b) in enumerate(ch0):
                    nc.tensor.matmul(
                        ramp_pss[t][i][:],
                        wts[t][:, k * 128:(k + 1) * 128],
                        x0k[k][:, a:b],
                        start=(seen[t] == 1),
                        stop=(seen[t] == KC1),
                    )
            for t in range(RAMP):
                for i, (a, b) in enumerate(ch0):
                    nc.scalar.activation(
                        ehs[t][:, a:b], ramp_pss[t][i][:],
                        AF.Relu, bias=eb1t[:, t:t + 1],
                    )

            for t in range(RAMP, KC2):
                wt = wts[t]
                cap = caps[t]
                ch = _chunks(cap)
                pss = [pp.tile([128, b - a], f32, tag="mm1", bufs=8,
                               name=f"ps1_{t}_{i}") for i, (a, b) in
                       enumerate(ch)]
                for i, (a, b) in enumerate(ch):
                    for k in range(KC1):
                        nc.tensor.matmul(
                            pss[i][:],
                            wt[:, k * 128:(k + 1) * 128],
                            xsrc(t, k, a, b),
                            start=(k == 0),
                            stop=(k == KC1 - 1),
                        )
                for i, (a, b) in enumerate(ch):
                    nc.scalar.activation(
                        ehs[t][:, a:b], pss[i][:],
                        AF.Relu, bias=eb1t[:, t:t + 1],
                    )
                if t == HK + 1:
                    # slot0's eh tiles are complete; its mm2 hides under
                    # the remaining slot1 mm1 work
                    emit_mm2_g0()
                if t == KC2 - 5:
                    # pre-accumulate the first half of slot1's mm2 so
                    # only 4 matmuls per out-tile trail the last mm1 tile
                    emit_mm2_g1(list(range(4)), finish=False)
            emit_mm2_g1(list(range(4, HK)), finish=True)

    return nc


def _route_host(x, rW1, rb1, rW2, rb2):
    """Exact router on host: top-2 expert ids + normalized weights."""
    xf = np.asarray(x, np.float64).reshape(B, DIN)
    rh = np.maximum(xf @ np.asarray(rW1, np.float64)
                    + np.asarray(rb1, np.float64), 0.0)
    logits = rh @ np.asarray(rW2, np.float64) + np.asarray(rb2, np.float64)
    m = logits.max(-1, keepdims=True)
    p = np.exp(logits - m)
    p /= p.sum(-1, keepdims=True)
    idx = np.argsort(-p, axis=-1, kind="stable")[:, :TOPK]
    w = np.take_along_axis(p, idx, axis=-1)
    w /= w.sum(-1, keepdims=True)
    return idx, w


def _plan(counts):
    """Pack the 16 expert-halves into 8 (slot0, slot1) core pairs.

    Returns (C0, C1, plan) with plan[c] = [expert_slot0, expert_slot1];
    core c takes half c%2 of each. Slot0 = the 4 largest experts, slot1 =
    the 4 smallest: provably minimal C0 + C1 for this slot structure.
    """
    order = np.argsort(-counts, kind="stable")
    C0 = int(counts[order[0]])
    C1 = max(int(counts[order[4]]), 1)
    plan = []
    for c in range(E):
        plan.append((int(order[c // 2]), int(order[4 + c // 2])))
    return C0, C1, plan


def _prep_inputs(x, rW1, rb1, rW2, rb2, eW1, eb1, eW2, eb2):
    """Host routing + shard/layout prep. Returns (in_maps, scatter, ntt)."""
    idx, w = _route_host(x, rW1, rb1, rW2, rb2)
    tok_of = [np.where((idx == e).any(-1))[0] for e in range(E)]
    w_of = [w[(t := tok_of[e]), (idx[t] == e).argmax(-1)]
            for e in range(E)]
    counts = np.array([len(t) for t in tok_of])
    C0, C1, plan = _plan(counts)

    xf = np.ascontiguousarray(np.asarray(x, np.float32).reshape(B, DIN))
    # [i, k, n] layout: xt[:, k, t] = xf[t, 128k + i]
    xt = xf.reshape(B, KC1, 128).transpose(2, 1, 0).astype(BF16)

    ew1_full = [np.asarray(eW1[e], np.float32)
                .reshape(KC1, 128, KC2, 128)
                .transpose(2, 1, 0, 3)
                .reshape(KC2, 128, DIN)
                .astype(BF16) for e in range(E)]
    eb1_full = [np.asarray(eb1[e], np.float32).reshape(KC2, 128).T
                for e in range(E)]
    ew2_full = [np.asarray(eW2[e], np.float32)
                .reshape(KC2, 128, NCLS)
                .transpose(1, 0, 2)
                .astype(BF16) for e in range(E)]

    pads = [-(-C0 // 128) * 128, -(-C1 // 128) * 128]
    ntt = (pads[0] + pads[1]) // 128
    caps = [C0, C1]

    in_maps = []
    scatter = []   # per core: (row_offset, token_ids) per slot
    for c in range(E):
        xblob = [np.zeros((128, KC1, C0), BF16),
                 np.zeros((128, KC1, C1), BF16)]
        ew1b = np.empty((KC2, 128, DIN), BF16)
        eb1b = np.empty((128, KC2), np.float32)
        ew2b = np.empty((128, KC2, NCLS), BF16)
        eb2b = np.empty((128, 2, NCLS), np.float32)
        wb = np.zeros((128, ntt), np.float32)
        half = c % 2
        sc = []
        for g in range(2):
            e = plan[c][g]
            toks = tok_of[e]
            n = len(toks)
            xblob[g][:, :, :n] = xt[:, :, toks]
            mlo = half * HK
            ew1b[g * HK:(g + 1) * HK] = ew1_full[e][mlo:mlo + HK]
            eb1b[:, g * HK:(g + 1) * HK] = eb1_full[e][:, mlo:mlo + HK]
            ew2b[:, g * HK:(g + 1) * HK] = ew2_full[e][:, mlo:mlo + HK]
            eb2b[:, g, :] = np.asarray(eb2[e], np.float32)[None, :]
            base_tt = 0 if g == 0 else pads[0] // 128
            wcol = np.zeros(pads[g], np.float32)
            wcol[:n] = w_of[e]
            wb[:, base_tt:base_tt + pads[g] // 128] = (
                wcol.reshape(-1, 128).T)
            j = np.arange(n)
            if g == 0:
                rows = base_tt * 128 + j
            else:
                # slot1 partials leave the core via one merged DMA with
                # rows interleaved as ntt1*token + tile
                ntt1 = pads[1] // 128
                rows = base_tt * 128 + (j % 128) * ntt1 + j // 128
            sc.append((rows, toks))
            assert n <= caps[g]
        in_maps.append({
            "x0": np.ascontiguousarray(xblob[0]),
            "x1": np.ascontiguousarray(xblob[1]),
            "ew1": np.ascontiguousarray(ew1b),
            "eb1": eb1b, "ew2": np.ascontiguousarray(ew2b),
            "eb2": eb2b, "w": wb,
        })
        scatter.append(sc)
    return in_maps, scatter, C0, C1


def kernel(x, rW1, rb1, rW2, rb2, eW1, eb1, eW2, eb2):
    global LAST_RESULTS
    _ensure_axon_profile_hook()
    from concourse.bass_utils import run_bass_kernel_spmd

    in_maps, scatter, C0, C1 = _prep_inputs(
        x, rW1, rb1, rW2, rb2, eW1, eb1, eW2, eb2)
    key = (C0, C1)
    nc = _PROGRAMS.get(key)
    if nc is None:
        nc = _build_program(C0, C1)
        nc.finalize()
        _PROGRAMS[key] = nc
    res = run_bass_kernel_spmd(nc, in_maps, core_ids=list(range(E)))
    LAST_RESULTS = res
    out = np.zeros((B, NCLS), np.float32)
    for c, r in enumerate(res.results):
        part = np.asarray(r["out"], np.float32)
        for (rows, toks) in scatter[c]:
            np.add.at(out, toks, part[rows])
    return out
